# revision 26
# baseline (speedup 1.0000x reference)
"""Trainium2 Bass kernel for DisentangleStaticNoiseLoss (NT-Xent style loss).

Math (matches the jax reference):
    x   : [K=8192, D=128] stacked embeddings (N=8 blocks of BS=1024)
    z   : row-normalized x;  S = (z @ z.T) / 0.5;  E = exp(S)
    row i (block b, sample r): positives = S[i, r + b'*BS] for b' != b,
    negatives = all j with j % BS != r.
    loss = mean over (i, pos) of [log(exp(pos) + sum_neg exp(neg)) - pos]

Sharding (data-parallel over rows, symmetric halving of the exp work):
E is symmetric, so the K x K matrix is computed only once per unordered
block pair instead of twice.  Each core receives the FULL z rotated so its
own 1024 rows come first (host-side np.roll), making the SPMD program
identical on every core.  Core c computes, for its rows, the local column
blocks B0..B3 fully plus the upper-triangular half of B4 (ragged by
128-row m-tile).  Block-pair coverage over all 8 cores:
  - difference d=1,2,3 pairs: covered once via B1..B3,
  - d=5,6,7 pairs: same unordered pairs, covered from the other side,
  - d=4 pairs: each of the two cores computes one triangle of the pair
    block; the 128x128 pair-diagonal subtiles are computed fully by BOTH
    cores, and a per-row correction column (their full row-sum) removes
    the double count at assembly time.
Per core the device emits:
  - rows_out [128,24] f32: per-(m-tile, slice) row sums of E (ACT accum),
  - cols_out [8,512]  f32: column sums of E over the core's rows for local
    columns 1024..5119 (PE ones-matmul into one PSUM bank, chunk k in
    PSUM partition k, accumulated across m-tiles),
  - g_out [128,48] f32: per-m-tile diagonals of B0..B4 (the positive-pair
    exp values + the self term) and the B4 double-count correction.
The host (the gather/unshard step) sums the row/col contributions into
F_i = sum_j E_ij, forms A_i = F_i - sum_b exp(pos), and reduces the
final scalar loss in float64: mean(log(exp(p)+A) - p).  This is the
all-reduce of the sharded partial sums; an on-device NRT collective
costs ~15-28us constant overhead, more than the entire saving.

Engine budget per core: ACT exps 128x37376 elements in 24 slices
(~49us, the bottleneck), PE ~28us (S matmuls + colsum matmuls), DVE
~10us (diag extracts + corr), one Pool copy, DMA 2MB in / 44KB out.
"""

import sys

import numpy as np

if "/opt/trn_rl_repo" not in sys.path:
    sys.path.insert(0, "/opt/trn_rl_repo")

N = 8
BS = 1024
D = 128
K = N * BS          # 8192
NCORES = 8
TEMP_SCALE = 2.0    # 1 / temperature

_NC_CACHE = {}


def _slice_sizes(m):
    """Per-m-tile activation slice widths over the 4096+w column stream."""
    w = 1024 - 128 * m
    total = 4096 + w
    if m % 2 == 0:
        sl = [2048, 1536, total - 3584]
    else:
        sl = [1536, 2048, total - 3584]
    assert sl[2] >= 128 and sl[2] <= 2048
    return sl


def _build_nc():
    import concourse.bacc as bacc
    import concourse.tile as tile
    from concourse import mybir

    f32 = mybir.dt.float32
    bf16 = mybir.dt.bfloat16
    AX = mybir.AxisListType
    OP = mybir.AluOpType
    AF = mybir.ActivationFunctionType

    nc = bacc.Bacc("TRN2", target_bir_lowering=False, debug=False)
    zt = nc.declare_dram_parameter("zt", [128, K], bf16, isOutput=False)
    ident = nc.declare_dram_parameter("ident", [128, 128], bf16, isOutput=False)
    # ind[:, 8k:8k+8] is the [128,8] indicator matrix with column k all-ones:
    # used as matmul lhsT it lands chunk k's column sums in PSUM partition k
    # (and adds zeros to the other partitions of the shared strip tile).
    ind = nc.declare_dram_parameter("ind", [128, 64], bf16, isOutput=False)
    rows_out = nc.declare_dram_parameter("rows_out", [128, 24], f32, isOutput=True)
    g_out = nc.declare_dram_parameter("g_out", [128, 48], f32, isOutput=True)
    cols_out = nc.declare_dram_parameter("cols_out", [8, 512], f32, isOutput=True)

    with tile.TileContext(nc) as tc:
        with (
            tc.tile_pool(name="persist", bufs=1) as P,
            tc.tile_pool(name="work", bufs=2) as W,
            tc.tile_pool(name="pmm", bufs=1, space="PSUM") as PM,
        ):
            # persistent SBUF.  zt lands in four 2048-col chunks issued from
            # the SP queue (gpsimd DMA triggers cost ~650ns of Q7 descriptor
            # generation EACH and serialize); the first chunk is all the
            # first m-tile needs, so compute starts as soon as it lands.
            ztg = [
                P.tile([128, 2048], bf16, tag=f"ztg{g}", name=f"ztg{g}")
                for g in range(4)
            ]
            idsb = P.tile([128, 128], bf16, tag="idsb")
            indsb = P.tile([128, 64], bf16, tag="indsb")
            # split the DMA triggers across two queues: each config costs
            # ~565ns of sequencer time, so one queue would serialize them
            for g in range(2):
                nc.sync.dma_start(out=ztg[g][:], in_=zt[:, g * 2048 : (g + 1) * 2048])
            nc.scalar.dma_start(out=idsb[:], in_=ident[:, :])
            nc.scalar.dma_start(out=indsb[:], in_=ind[:, :])
            for g in range(2, 4):
                nc.scalar.dma_start(
                    out=ztg[g][:], in_=zt[:, g * 2048 : (g + 1) * 2048]
                )
            acc = P.tile([128, 72], f32, tag="acc")  # rows 0:24, g 24:72

            # PSUM: two S tiles (double buffer) + colsum strip bank
            sA = PM.tile([128, 2048], f32, tag="sA")
            sB = PM.tile([128, 1536], f32, tag="sB")
            CS = PM.tile([128, 512], f32, tag="cs")

            # warm the PE p-state ramp while the zt DMA is in flight (the
            # ramp needs continuous execution; the real fills continue the
            # stream).  The matmul results are discarded.
            wtile = P.tile([128, 512], bf16, tag="wtile")
            nc.gpsimd.memset(wtile[:], 0.03)
            for _ in range(4):
                nc.tensor.matmul(
                    sB[:, 0:512], wtile[:, 0:128], wtile[:], start=True, stop=True
                )

            def rhs_ap(c0, c1):
                """zt SBUF AP for local column range [c0, c1) (single chunk)."""
                g = c0 // 2048
                assert (c1 - 1) // 2048 == g
                return ztg[g][:, c0 - g * 2048 : c1 - g * 2048]

            def emit_colsums(m, E, win=None):
                # colsum matmuls: local cols 1024..4096+w, chunk k of 512 cols
                # accumulated across m into PSUM partition k of CS via the
                # indicator lhsT (adds zeros to the other partitions).  win
                # restricts to chunks within a stream window (m=7 per-slice).
                for k in range(8):
                    c0 = 1024 + 512 * k
                    c1 = c0 + 512
                    if k < 6:
                        j0, j1 = 0, 512
                        st0 = c0
                    else:
                        # B4 chunks: valid cols are >= 4096+128m
                        lo = max(c0, 4096 + 128 * m)
                        if lo >= c1:
                            continue
                        j0, j1 = lo - c0, 512
                        st0 = lo - 128 * m  # stream position
                    if win is not None and not (win[0] <= st0 and st0 + (j1 - j0) <= win[1]):
                        continue
                    nc.tensor.matmul(
                        CS[0:8, j0:j1],
                        indsb[:, 8 * k : 8 * k + 8],
                        E[:, st0 : st0 + (j1 - j0)],
                        start=(m == 0 and k == 0),
                        stop=(m == 7 and k == 7),
                        skip_group_check=True,
                    )

            def emit_extract(m, E, k):
                # diag extract: positives + self term (stream pos k*1024+m*128
                # for k<4; B4 pair-diag subtile sits at stream 4096)
                cs0 = k * 1024 + m * 128 if k < 4 else 4096
                gc = 24 + 6 * m + k
                scr = W.tile([128, 128], bf16, tag="scr")
                nc.vector.scalar_tensor_tensor(
                    out=scr[:],
                    in0=E[:, cs0 : cs0 + 128],
                    scalar=1.0,
                    in1=idsb[:],
                    op0=OP.mult,
                    op1=OP.mult,
                    accum_out=acc[:, gc : gc + 1],
                )

            def emit_corr(m, E):
                # corr: full row-sum of the B4 pair-diagonal subtile
                # (gpsimd tensor_reduce is partition-axis only, so DVE)
                nc.vector.tensor_reduce(
                    out=acc[:, 24 + 6 * m + 5 : 24 + 6 * m + 6],
                    in_=E[:, 4096 : 4096 + 128],
                    axis=AX.X,
                    op=OP.add,
                )

            pending = None  # (m, E) whose colsums are issued one m-tile late
            for m in range(8):
                w = 1024 - 128 * m
                lhsT = ztg[0][:, m * 128 : (m + 1) * 128]
                E = W.tile([128, 5120], bf16, tag="E", bufs=3)
                sl = _slice_sizes(m)
                # the two large slices keep the ACT accumulator row sum; only
                # the small trailing slice rides DVE (which also carries the
                # diag extracts -- keep it clearly under the ACT streak)
                act_si = (0, 1)
                so = 0  # stream offset
                for si, L in enumerate(sl):
                    ps = sA if (si + m) % 2 == 0 else sB
                    # fill PSUM with S via 512-col matmul chunks
                    q0 = 0
                    while q0 < L:
                        qw = min(512, L - q0)
                        s_pos = so + q0
                        # local col of stream position
                        if s_pos < 4096:
                            c0 = s_pos
                        else:
                            c0 = s_pos + 128 * m
                        nc.tensor.matmul(
                            ps[:, q0 : q0 + qw],
                            lhsT,
                            rhs_ap(c0, c0 + qw),
                            start=True,
                            stop=True,
                        )
                        q0 += qw
                    nc.scalar.activation(
                        out=E[:, so : so + L],
                        in_=ps[:, 0:L],
                        func=AF.Exp,
                        scale=TEMP_SCALE,
                        **(
                            {"accum_out": acc[:, 3 * m + si : 3 * m + si + 1]}
                            if si in act_si
                            else {}
                        ),
                    )
                    if si not in act_si:
                        # row sums for the smaller slices ride on DVE instead
                        # of costing ACT an accumulator read
                        nc.vector.tensor_reduce(
                            out=acc[:, 3 * m + si : 3 * m + si + 1],
                            in_=E[:, so : so + L],
                            axis=AX.X,
                            op=OP.add,
                        )
                    if m == 7:
                        # final m-tile: drain everything per-slice to shrink
                        # the serial tail; m=6's colsums slot in right after
                        # the first fill so they stay off the tail
                        if si == 0 and pending is not None:
                            emit_colsums(*pending)
                            pending = None
                        for k in range(5):
                            cs0 = k * 1024 + m * 128 if k < 4 else 4096
                            if so <= cs0 and cs0 + 128 <= so + L:
                                emit_extract(m, E, k)
                        if so <= 4096 < so + L:
                            emit_corr(m, E)
                        emit_colsums(m, E, win=(so, so + L))
                    so += L

                if m < 7:
                    for k in range(5):
                        emit_extract(m, E, k)
                    emit_corr(m, E)
                    # previous m-tile's colsums go on the PE queue here, so
                    # they execute in the shadow of this m-tile's exps instead
                    # of delaying the next m-tile's PSUM fills
                    if pending is not None:
                        emit_colsums(*pending)
                    pending = (m, E)

            # drain CS -> SBUF -> DRAM; ship accumulators
            cs_sb = P.tile([8, 512], f32, tag="cs_sb")
            nc.vector.tensor_copy(out=cs_sb[:], in_=CS[0:8, :])
            nc.sync.dma_start(out=rows_out[:, :], in_=acc[:, 0:24])
            nc.sync.dma_start(out=g_out[:, :], in_=acc[:, 24:72])
            nc.sync.dma_start(out=cols_out[:, :], in_=cs_sb[:])

    nc.compile()
    return nc


def _get_nc():
    if "nc" not in _NC_CACHE:
        _NC_CACHE["nc"] = _build_nc()
    return _NC_CACHE["nc"]


def _host_prep(sim):
    import ml_dtypes

    x = np.asarray(sim, dtype=np.float64).reshape(K, D)
    z = (x / np.maximum(np.linalg.norm(x, axis=1, keepdims=True), 1e-8)).astype(
        np.float32
    )
    ident = np.eye(128, dtype=ml_dtypes.bfloat16)
    ind = np.zeros((128, 64), dtype=ml_dtypes.bfloat16)
    for k in range(8):
        ind[:, 8 * k + k] = 1
    in_maps = []
    for c in range(NCORES):
        ztc = np.ascontiguousarray(
            np.roll(z, -c * BS, axis=0).T.astype(ml_dtypes.bfloat16)
        )
        in_maps.append({"zt": ztc, "ident": ident, "ind": ind})
    return in_maps


def _assemble(results):
    """Gather/unshard: combine per-core partial sums into the scalar loss."""
    F = np.zeros(K, np.float64)
    G = np.zeros((K, 8), np.float64)
    li = np.arange(128)
    for c in range(NCORES):
        rows = np.asarray(results[c]["rows_out"], np.float64)
        g = np.asarray(results[c]["g_out"], np.float64)
        cols = np.asarray(results[c]["cols_out"], np.float64)
        for m in range(8):
            gr = c * BS + m * 128 + li  # global rows
            F[gr] += rows[:, 3 * m : 3 * m + 3].sum(1)
            F[gr] -= g[:, 6 * m + 5]  # B4 diag-subtile double count
            for k in range(5):
                bc = (c + k) % 8
                G[gr, bc] = g[:, 6 * m + k]
                G[bc * BS + m * 128 + li, c] = g[:, 6 * m + k]
        for k in range(8):
            gcols = (c * BS + 1024 + 512 * k + np.arange(512)) % K
            F[gcols] += cols[k]
    P = G.sum(1)
    A = F - P
    idx = np.arange(K)
    mask = np.ones((K, 8), bool)
    mask[idx, idx // BS] = False
    Epos = G[mask].reshape(K, 7)
    L = np.log(Epos + A[:, None]) - np.log(Epos)
    return np.float32(L.sum() / (K * 7))


def kernel(sim: np.ndarray, _want_results: bool = False, _trace: bool = False):
    in_maps = _host_prep(sim)
    nc = _get_nc()
    from concourse.bass_utils import run_bass_kernel_spmd

    res = run_bass_kernel_spmd(nc, in_maps, list(range(NCORES)), trace=_trace)
    loss = _assemble(res.results)
    if _want_results:
        return loss, res
    return loss


if __name__ == "__main__":
    nc = _build_nc()
    print("build OK")


# revision 32
# speedup vs baseline: 1.0295x; 1.0295x over previous
"""Trainium2 Bass kernel for DisentangleStaticNoiseLoss (NT-Xent style loss).

Math (matches the jax reference):
    x   : [K=8192, D=128] stacked embeddings (N=8 blocks of BS=1024)
    z   : row-normalized x;  S = (z @ z.T) / 0.5;  E = exp(S)
    row i (block b, sample r): positives = S[i, r + b'*BS] for b' != b,
    negatives = all j with j % BS != r.
    loss = mean over (i, pos) of [log(exp(pos) + sum_neg exp(neg)) - pos]

Sharding (data-parallel over rows, symmetric halving of the exp work):
E is symmetric, so the K x K matrix is computed only once per unordered
block pair instead of twice.  Each core receives the FULL z rotated so its
own 1024 rows come first (host-side np.roll), making the SPMD program
identical on every core.  Core c computes, for its rows, the local column
blocks B0..B3 fully plus the upper-triangular half of B4 (ragged by
128-row m-tile).  Block-pair coverage over all 8 cores:
  - difference d=1,2,3 pairs: covered once via B1..B3,
  - d=5,6,7 pairs: same unordered pairs, covered from the other side,
  - d=4 pairs: each of the two cores computes one triangle of the pair
    block; the 128x128 pair-diagonal subtiles are computed fully by BOTH
    cores, and a per-row correction column (their full row-sum) removes
    the double count at assembly time.
Per core the device emits:
  - rows_out [128,24] f32: per-(m-tile, slice) row sums of E (ACT accum),
  - cols_out [8,512]  f32: column sums of E over the core's rows for local
    columns 1024..5119 (PE ones-matmul into one PSUM bank, chunk k in
    PSUM partition k, accumulated across m-tiles),
  - g_out [128,48] f32: per-m-tile diagonals of B0..B4 (the positive-pair
    exp values + the self term) and the B4 double-count correction.
The host (the gather/unshard step) sums the row/col contributions into
F_i = sum_j E_ij, forms A_i = F_i - sum_b exp(pos), and reduces the
final scalar loss in float64: mean(log(exp(p)+A) - p).  This is the
all-reduce of the sharded partial sums; an on-device NRT collective
costs ~15-28us constant overhead, more than the entire saving.

Engine budget per core: ACT exps 128x37376 elements in 24 slices
(~49us, the bottleneck), PE ~28us (S matmuls + colsum matmuls), DVE
~10us (diag extracts + corr), one Pool copy, DMA 2MB in / 44KB out.
"""

import sys

import numpy as np

if "/opt/trn_rl_repo" not in sys.path:
    sys.path.insert(0, "/opt/trn_rl_repo")

N = 8
BS = 1024
D = 128
K = N * BS          # 8192
NCORES = 8
TEMP_SCALE = 2.0    # 1 / temperature

_NC_CACHE = {}


def _slice_sizes(m):
    """Per-m-tile activation slice widths over the 4096+w column stream.
    m=0 starts with 512-wide slices so the first exps only wait on the
    first 512-col zt sub-DMA instead of the whole first chunk."""
    w = 1024 - 128 * m
    total = 4096 + w
    if m == 0:
        sl = [512, 512, 512, 512, 2048, 1024]
    elif m % 2 == 1:
        sl = [2048, 1536, total - 3584]
    else:
        sl = [1536, 2048, total - 3584]
    assert sum(sl) == total and all(128 <= L <= 2048 for L in sl)
    return sl


SLICES = [_slice_sizes(m) for m in range(8)]
ROFF = [sum(len(SLICES[i]) for i in range(m)) for m in range(8)]
NSL = sum(len(s) for s in SLICES)  # 27 row-sum columns
ZLOC = 5120  # local columns actually used (B0..B4); the rest is never read


def _build_nc():
    import concourse.bacc as bacc
    import concourse.tile as tile
    from concourse import mybir

    f32 = mybir.dt.float32
    bf16 = mybir.dt.bfloat16
    AX = mybir.AxisListType
    OP = mybir.AluOpType
    AF = mybir.ActivationFunctionType

    nc = bacc.Bacc("TRN2", target_bir_lowering=False, debug=False)
    zt = nc.declare_dram_parameter("zt", [128, ZLOC], bf16, isOutput=False)
    ident = nc.declare_dram_parameter("ident", [128, 128], bf16, isOutput=False)
    # ind[:, 8k:8k+8] is the [128,8] indicator matrix with column k all-ones:
    # used as matmul lhsT it lands chunk k's column sums in PSUM partition k
    # (and adds zeros to the other partitions of the shared strip tile).
    ind = nc.declare_dram_parameter("ind", [128, 64], bf16, isOutput=False)
    rows_out = nc.declare_dram_parameter("rows_out", [128, NSL], f32, isOutput=True)
    g_out = nc.declare_dram_parameter("g_out", [128, 48], f32, isOutput=True)
    cols_out = nc.declare_dram_parameter("cols_out", [8, 512], f32, isOutput=True)

    with tile.TileContext(nc) as tc:
        with (
            tc.tile_pool(name="persist", bufs=1) as P,
            tc.tile_pool(name="work", bufs=2) as W,
            tc.tile_pool(name="pmm", bufs=1, space="PSUM") as PM,
        ):
            # persistent SBUF.  Only local cols 0..5119 are ever read.  The
            # first 2048 cols land as four 512-col sub-DMAs so the first
            # m-tile's 512-wide exps start as soon as each lands; each DMA
            # trigger costs ~0.6us of sequencer time so they are split
            # across the SP and ACT queues.
            z0s = [
                P.tile([128, 512], bf16, tag=f"z0s{g}", name=f"z0s{g}")
                for g in range(4)
            ]
            ztg1 = P.tile([128, 2048], bf16, tag="ztg1")
            ztg2 = P.tile([128, 1024], bf16, tag="ztg2")
            idsb = P.tile([128, 128], bf16, tag="idsb")
            indsb = P.tile([128, 64], bf16, tag="indsb")
            for g in range(4):
                nc.sync.dma_start(
                    out=z0s[g][:], in_=zt[:, g * 512 : (g + 1) * 512]
                )
            nc.scalar.dma_start(out=ztg1[:], in_=zt[:, 2048:4096])
            nc.scalar.dma_start(out=idsb[:], in_=ident[:, :])
            nc.scalar.dma_start(out=indsb[:], in_=ind[:, :])
            nc.scalar.dma_start(out=ztg2[:], in_=zt[:, 4096:5120])
            acc = P.tile([128, NSL + 48], f32, tag="acc")  # rows, then g

            # PSUM: two S tiles (double buffer) + colsum strip bank
            sA = PM.tile([128, 2048], f32, tag="sA")
            sB = PM.tile([128, 1536], f32, tag="sB")
            CS = PM.tile([128, 512], f32, tag="cs")

            # warm the PE p-state ramp while the zt DMA is in flight (the
            # ramp needs continuous execution; the real fills continue the
            # stream).  The matmul results are discarded.
            wtile = P.tile([128, 512], bf16, tag="wtile")
            nc.gpsimd.memset(wtile[:], 0.03)
            for _ in range(4):
                nc.tensor.matmul(
                    sB[:, 0:512], wtile[:, 0:128], wtile[:], start=True, stop=True
                )

            def rhs_ap(c0, c1):
                """zt SBUF AP for local column range [c0, c1) (single tile)."""
                if c0 < 2048:
                    g = c0 // 512
                    assert (c1 - 1) // 512 == g
                    return z0s[g][:, c0 - g * 512 : c1 - g * 512]
                if c0 < 4096:
                    assert c1 <= 4096
                    return ztg1[:, c0 - 2048 : c1 - 2048]
                assert c1 <= 5120
                return ztg2[:, c0 - 4096 : c1 - 4096]

            def emit_colsums(m, E, win=None):
                # colsum matmuls: local cols 1024..4096+w, chunk k of 512 cols
                # accumulated across m into PSUM partition k of CS via the
                # indicator lhsT (adds zeros to the other partitions).  win
                # restricts to chunks within a stream window (m=7 per-slice).
                for k in range(8):
                    c0 = 1024 + 512 * k
                    c1 = c0 + 512
                    if k < 6:
                        j0, j1 = 0, 512
                        st0 = c0
                    else:
                        # B4 chunks: valid cols are >= 4096+128m
                        lo = max(c0, 4096 + 128 * m)
                        if lo >= c1:
                            continue
                        j0, j1 = lo - c0, 512
                        st0 = lo - 128 * m  # stream position
                    if win is not None and not (win[0] <= st0 and st0 + (j1 - j0) <= win[1]):
                        continue
                    nc.tensor.matmul(
                        CS[0:8, j0:j1],
                        indsb[:, 8 * k : 8 * k + 8],
                        E[:, st0 : st0 + (j1 - j0)],
                        start=(m == 0 and k == 0),
                        stop=(m == 7 and k == 7),
                        skip_group_check=True,
                    )

            def emit_extract(m, E, k):
                # diag extract: positives + self term (stream pos k*1024+m*128
                # for k<4; B4 pair-diag subtile sits at stream 4096)
                cs0 = k * 1024 + m * 128 if k < 4 else 4096
                gc = NSL + 6 * m + k
                scr = W.tile([128, 128], bf16, tag="scr")
                nc.vector.scalar_tensor_tensor(
                    out=scr[:],
                    in0=E[:, cs0 : cs0 + 128],
                    scalar=1.0,
                    in1=idsb[:],
                    op0=OP.mult,
                    op1=OP.mult,
                    accum_out=acc[:, gc : gc + 1],
                )

            def emit_corr(m, E):
                # corr: full row-sum of the B4 pair-diagonal subtile
                # (gpsimd tensor_reduce is partition-axis only, so DVE)
                nc.vector.tensor_reduce(
                    out=acc[:, NSL + 6 * m + 5 : NSL + 6 * m + 6],
                    in_=E[:, 4096 : 4096 + 128],
                    axis=AX.X,
                    op=OP.add,
                )

            pending = None  # (m, E) whose colsums are issued one m-tile late
            for m in range(8):
                w = 1024 - 128 * m
                lt = (m * 128) // 512
                lhsT = z0s[lt][:, m * 128 - lt * 512 : (m + 1) * 128 - lt * 512]
                E = W.tile([128, 5120], bf16, tag="E", bufs=3)
                sl = SLICES[m]
                # all but the last slice keep the ACT accumulator row sum;
                # the trailing slice rides DVE (which also carries the diag
                # extracts -- keep it clearly under the ACT streak)
                act_si = tuple(range(len(sl) - 1))
                so = 0  # stream offset
                for si, L in enumerate(sl):
                    ps = sA if (ROFF[m] + si) % 2 == 0 else sB
                    # fill PSUM with S via 512-col matmul chunks
                    q0 = 0
                    while q0 < L:
                        qw = min(512, L - q0)
                        s_pos = so + q0
                        # local col of stream position
                        if s_pos < 4096:
                            c0 = s_pos
                        else:
                            c0 = s_pos + 128 * m
                        nc.tensor.matmul(
                            ps[:, q0 : q0 + qw],
                            lhsT,
                            rhs_ap(c0, c0 + qw),
                            start=True,
                            stop=True,
                        )
                        q0 += qw
                    nc.scalar.activation(
                        out=E[:, so : so + L],
                        in_=ps[:, 0:L],
                        func=AF.Exp,
                        scale=TEMP_SCALE,
                        **(
                            {"accum_out": acc[:, ROFF[m] + si : ROFF[m] + si + 1]}
                            if si in act_si
                            else {}
                        ),
                    )
                    if si not in act_si:
                        # row sums for the smaller slices ride on DVE instead
                        # of costing ACT an accumulator read
                        nc.vector.tensor_reduce(
                            out=acc[:, ROFF[m] + si : ROFF[m] + si + 1],
                            in_=E[:, so : so + L],
                            axis=AX.X,
                            op=OP.add,
                        )
                    if m == 7:
                        # final m-tile: drain everything per-slice to shrink
                        # the serial tail; m=6's colsums slot in right after
                        # the first fill so they stay off the tail
                        if si == 0 and pending is not None:
                            emit_colsums(*pending)
                            pending = None
                        for k in range(5):
                            cs0 = k * 1024 + m * 128 if k < 4 else 4096
                            if so <= cs0 and cs0 + 128 <= so + L:
                                emit_extract(m, E, k)
                        if so <= 4096 < so + L:
                            emit_corr(m, E)
                        emit_colsums(m, E, win=(so, so + L))
                    so += L

                if m < 7:
                    for k in range(5):
                        emit_extract(m, E, k)
                    emit_corr(m, E)
                    # previous m-tile's colsums go on the PE queue here, so
                    # they execute in the shadow of this m-tile's exps instead
                    # of delaying the next m-tile's PSUM fills
                    if pending is not None:
                        emit_colsums(*pending)
                    pending = (m, E)

            # drain CS -> SBUF -> DRAM; ship accumulators
            cs_sb = P.tile([8, 512], f32, tag="cs_sb")
            nc.vector.tensor_copy(out=cs_sb[:], in_=CS[0:8, :])
            nc.sync.dma_start(out=rows_out[:, :], in_=acc[:, 0:NSL])
            nc.sync.dma_start(out=g_out[:, :], in_=acc[:, NSL : NSL + 48])
            nc.sync.dma_start(out=cols_out[:, :], in_=cs_sb[:])

    nc.compile()
    return nc


def _get_nc():
    if "nc" not in _NC_CACHE:
        _NC_CACHE["nc"] = _build_nc()
    return _NC_CACHE["nc"]


def _host_prep(sim):
    import ml_dtypes

    x = np.asarray(sim, dtype=np.float64).reshape(K, D)
    z = (x / np.maximum(np.linalg.norm(x, axis=1, keepdims=True), 1e-8)).astype(
        np.float32
    )
    ident = np.eye(128, dtype=ml_dtypes.bfloat16)
    ind = np.zeros((128, 64), dtype=ml_dtypes.bfloat16)
    for k in range(8):
        ind[:, 8 * k + k] = 1
    in_maps = []
    for c in range(NCORES):
        ztc = np.ascontiguousarray(
            np.roll(z, -c * BS, axis=0)[:ZLOC].T.astype(ml_dtypes.bfloat16)
        )
        in_maps.append({"zt": ztc, "ident": ident, "ind": ind})
    return in_maps


def _assemble(results):
    """Gather/unshard: combine per-core partial sums into the scalar loss."""
    F = np.zeros(K, np.float64)
    G = np.zeros((K, 8), np.float64)
    li = np.arange(128)
    for c in range(NCORES):
        rows = np.asarray(results[c]["rows_out"], np.float64)
        g = np.asarray(results[c]["g_out"], np.float64)
        cols = np.asarray(results[c]["cols_out"], np.float64)
        for m in range(8):
            gr = c * BS + m * 128 + li  # global rows
            F[gr] += rows[:, ROFF[m] : ROFF[m] + len(SLICES[m])].sum(1)
            F[gr] -= g[:, 6 * m + 5]  # B4 diag-subtile double count
            for k in range(5):
                bc = (c + k) % 8
                G[gr, bc] = g[:, 6 * m + k]
                G[bc * BS + m * 128 + li, c] = g[:, 6 * m + k]
        for k in range(8):
            gcols = (c * BS + 1024 + 512 * k + np.arange(512)) % K
            F[gcols] += cols[k]
    P = G.sum(1)
    A = F - P
    idx = np.arange(K)
    mask = np.ones((K, 8), bool)
    mask[idx, idx // BS] = False
    Epos = G[mask].reshape(K, 7)
    L = np.log(Epos + A[:, None]) - np.log(Epos)
    return np.float32(L.sum() / (K * 7))


def kernel(sim: np.ndarray, _want_results: bool = False, _trace: bool = False):
    in_maps = _host_prep(sim)
    nc = _get_nc()
    from concourse.bass_utils import run_bass_kernel_spmd

    res = run_bass_kernel_spmd(nc, in_maps, list(range(NCORES)), trace=_trace)
    loss = _assemble(res.results)
    if _want_results:
        return loss, res
    return loss


if __name__ == "__main__":
    nc = _build_nc()
    print("build OK")


# revision 33
# speedup vs baseline: 1.0449x; 1.0150x over previous
"""Trainium2 Bass kernel for DisentangleStaticNoiseLoss (NT-Xent style loss).

Math (matches the jax reference):
    x   : [K=8192, D=128] stacked embeddings (N=8 blocks of BS=1024)
    z   : row-normalized x;  S = (z @ z.T) / 0.5;  E = exp(S)
    row i (block b, sample r): positives = S[i, r + b'*BS] for b' != b,
    negatives = all j with j % BS != r.
    loss = mean over (i, pos) of [log(exp(pos) + sum_neg exp(neg)) - pos]

Sharding (data-parallel over rows, symmetric halving of the exp work):
E is symmetric, so the K x K matrix is computed only once per unordered
block pair instead of twice.  Each core receives the FULL z rotated so its
own 1024 rows come first (host-side np.roll), making the SPMD program
identical on every core.  Core c computes, for its rows, the local column
blocks B0..B3 fully plus the upper-triangular half of B4 (ragged by
128-row m-tile).  Block-pair coverage over all 8 cores:
  - difference d=1,2,3 pairs: covered once via B1..B3,
  - d=5,6,7 pairs: same unordered pairs, covered from the other side,
  - d=4 pairs: each of the two cores computes one triangle of the pair
    block; the 128x128 pair-diagonal subtiles are computed fully by BOTH
    cores, and a per-row correction column (their full row-sum) removes
    the double count at assembly time.
Per core the device emits:
  - rows_out [128,24] f32: per-(m-tile, slice) row sums of E (ACT accum),
  - cols_out [8,512]  f32: column sums of E over the core's rows for local
    columns 1024..5119 (PE ones-matmul into one PSUM bank, chunk k in
    PSUM partition k, accumulated across m-tiles),
  - g_out [128,48] f32: per-m-tile diagonals of B0..B4 (the positive-pair
    exp values + the self term) and the B4 double-count correction.
The host (the gather/unshard step) sums the row/col contributions into
F_i = sum_j E_ij, forms A_i = F_i - sum_b exp(pos), and reduces the
final scalar loss in float64: mean(log(exp(p)+A) - p).  This is the
all-reduce of the sharded partial sums; an on-device NRT collective
costs ~15-28us constant overhead, more than the entire saving.

Engine budget per core: ACT exps 128x37376 elements in 24 slices
(~49us, the bottleneck), PE ~28us (S matmuls + colsum matmuls), DVE
~10us (diag extracts + corr), one Pool copy, DMA 2MB in / 44KB out.
"""

import sys

import numpy as np

if "/opt/trn_rl_repo" not in sys.path:
    sys.path.insert(0, "/opt/trn_rl_repo")

N = 8
BS = 1024
D = 128
K = N * BS          # 8192
NCORES = 8
TEMP_SCALE = 2.0    # 1 / temperature

_NC_CACHE = {}


def _slice_sizes(m):
    """Per-m-tile activation slice widths over the 4096+w column stream.
    m=0 starts with 512-wide slices so the first exps only wait on the
    first 512-col zt sub-DMA instead of the whole first chunk."""
    w = 1024 - 128 * m
    total = 4096 + w
    if m == 0:
        sl = [512, 512, 512, 512, 2048, 1024]
    elif m % 2 == 1:
        sl = [2048, 1536, total - 3584]
    else:
        sl = [1536, 2048, total - 3584]
    assert sum(sl) == total and all(128 <= L <= 2048 for L in sl)
    return sl


SLICES = [_slice_sizes(m) for m in range(8)]
ROFF = [sum(len(SLICES[i]) for i in range(m)) for m in range(8)]
NSL = sum(len(s) for s in SLICES)  # 27 row-sum columns
ZLOC = 5120  # local columns actually used (B0..B4); the rest is never read


def _build_nc():
    import concourse.bacc as bacc
    import concourse.tile as tile
    from concourse import mybir

    f32 = mybir.dt.float32
    bf16 = mybir.dt.bfloat16
    AX = mybir.AxisListType
    OP = mybir.AluOpType
    AF = mybir.ActivationFunctionType

    nc = bacc.Bacc("TRN2", target_bir_lowering=False, debug=False)
    zt = nc.declare_dram_parameter("zt", [128, ZLOC], bf16, isOutput=False)
    ident = nc.declare_dram_parameter("ident", [128, 128], bf16, isOutput=False)
    # ind[:, 8k:8k+8] is the [128,8] indicator matrix with column k all-ones:
    # used as matmul lhsT it lands chunk k's column sums in PSUM partition k
    # (and adds zeros to the other partitions of the shared strip tile).
    ind = nc.declare_dram_parameter("ind", [128, 64], bf16, isOutput=False)
    rows_out = nc.declare_dram_parameter("rows_out", [128, NSL], f32, isOutput=True)
    g_out = nc.declare_dram_parameter("g_out", [128, 48], f32, isOutput=True)
    cols_out = nc.declare_dram_parameter("cols_out", [8, 512], f32, isOutput=True)

    with tile.TileContext(nc) as tc:
        with (
            tc.tile_pool(name="persist", bufs=1) as P,
            tc.tile_pool(name="work", bufs=2) as W,
            tc.tile_pool(name="pmm", bufs=1, space="PSUM") as PM,
        ):
            # persistent SBUF.  Only local cols 0..5119 are ever read.  The
            # first 2048 cols land as four 512-col sub-DMAs so the first
            # m-tile's 512-wide exps start as soon as each lands; each DMA
            # trigger costs ~0.6us of sequencer time so they are split
            # across the SP and ACT queues.
            z0s = [
                P.tile([128, 512], bf16, tag=f"z0s{g}", name=f"z0s{g}")
                for g in range(4)
            ]
            ztg1 = P.tile([128, 2048], bf16, tag="ztg1")
            ztg2 = P.tile([128, 1024], bf16, tag="ztg2")
            idsb = P.tile([128, 128], bf16, tag="idsb")
            indsb = P.tile([128, 64], bf16, tag="indsb")
            for g in range(4):
                nc.sync.dma_start(
                    out=z0s[g][:], in_=zt[:, g * 512 : (g + 1) * 512]
                )
            nc.scalar.dma_start(out=ztg1[:], in_=zt[:, 2048:4096])
            nc.scalar.dma_start(out=idsb[:], in_=ident[:, :])
            nc.scalar.dma_start(out=indsb[:], in_=ind[:, :])
            nc.scalar.dma_start(out=ztg2[:], in_=zt[:, 4096:5120])
            acc = P.tile([128, NSL + 48], f32, tag="acc")  # rows, then g

            # PSUM: two S tiles (double buffer) + colsum strip bank
            sA = PM.tile([128, 2048], f32, tag="sA")
            sB = PM.tile([128, 1536], f32, tag="sB")
            CS = PM.tile([128, 512], f32, tag="cs")

            # warm the PE p-state ramp while the zt DMA is in flight (the
            # ramp needs continuous execution; the real fills continue the
            # stream).  The matmul results are discarded.
            wtile = P.tile([128, 512], bf16, tag="wtile")
            nc.gpsimd.memset(wtile[:], 0.03)
            for _ in range(4):
                nc.tensor.matmul(
                    sB[:, 0:512], wtile[:, 0:128], wtile[:], start=True, stop=True
                )

            def rhs_ap(c0, c1):
                """zt SBUF AP for local column range [c0, c1) (single tile)."""
                if c0 < 2048:
                    g = c0 // 512
                    assert (c1 - 1) // 512 == g
                    return z0s[g][:, c0 - g * 512 : c1 - g * 512]
                if c0 < 4096:
                    assert c1 <= 4096
                    return ztg1[:, c0 - 2048 : c1 - 2048]
                assert c1 <= 5120
                return ztg2[:, c0 - 4096 : c1 - 4096]

            def emit_colsums(m, E, win=None):
                # colsum matmuls: local cols 1024..4096+w, chunk k of 512 cols
                # accumulated across m into PSUM partition k of CS via the
                # indicator lhsT (adds zeros to the other partitions).  win
                # restricts to chunks within a stream window (m=7 per-slice).
                for k in range(8):
                    c0 = 1024 + 512 * k
                    c1 = c0 + 512
                    if k < 6:
                        j0, j1 = 0, 512
                        st0 = c0
                    else:
                        # B4 chunks: valid cols are >= 4096+128m
                        lo = max(c0, 4096 + 128 * m)
                        if lo >= c1:
                            continue
                        j0, j1 = lo - c0, 512
                        st0 = lo - 128 * m  # stream position
                    if win is not None and not (win[0] <= st0 and st0 + (j1 - j0) <= win[1]):
                        continue
                    nc.tensor.matmul(
                        CS[0:8, j0:j1],
                        indsb[:, 8 * k : 8 * k + 8],
                        E[:, st0 : st0 + (j1 - j0)],
                        start=(m == 0 and k == 0),
                        stop=(m == 7 and k == 7),
                        skip_group_check=True,
                    )

            def emit_extract(m, E, k):
                # diag extract: positives + self term (stream pos k*1024+m*128
                # for k<4; B4 pair-diag subtile sits at stream 4096)
                cs0 = k * 1024 + m * 128 if k < 4 else 4096
                gc = NSL + 6 * m + k
                scr = W.tile([128, 128], bf16, tag="scr")
                nc.vector.scalar_tensor_tensor(
                    out=scr[:],
                    in0=E[:, cs0 : cs0 + 128],
                    scalar=1.0,
                    in1=idsb[:],
                    op0=OP.mult,
                    op1=OP.mult,
                    accum_out=acc[:, gc : gc + 1],
                )

            def emit_corr(m, E):
                # corr: full row-sum of the B4 pair-diagonal subtile
                # (gpsimd tensor_reduce is partition-axis only, so DVE)
                nc.vector.tensor_reduce(
                    out=acc[:, NSL + 6 * m + 5 : NSL + 6 * m + 6],
                    in_=E[:, 4096 : 4096 + 128],
                    axis=AX.X,
                    op=OP.add,
                )

            pending = None  # (m, E) whose colsums are issued one m-tile late
            for m in range(8):
                w = 1024 - 128 * m
                lt = (m * 128) // 512
                lhsT = z0s[lt][:, m * 128 - lt * 512 : (m + 1) * 128 - lt * 512]
                E = W.tile([128, 5120], bf16, tag="E", bufs=3)
                sl = SLICES[m]
                # all but the last slice keep the ACT accumulator row sum;
                # the trailing slice rides DVE (which also carries the diag
                # extracts).  The final m-tile keeps everything on ACT so the
                # post-streak tail only carries the diag extracts.
                act_si = (
                    tuple(range(len(sl)))
                    if m == 7
                    else tuple(range(len(sl) - 1))
                )
                so = 0  # stream offset
                for si, L in enumerate(sl):
                    ps = sA if (ROFF[m] + si) % 2 == 0 else sB
                    # fill PSUM with S via 512-col matmul chunks
                    q0 = 0
                    while q0 < L:
                        qw = min(512, L - q0)
                        s_pos = so + q0
                        # local col of stream position
                        if s_pos < 4096:
                            c0 = s_pos
                        else:
                            c0 = s_pos + 128 * m
                        nc.tensor.matmul(
                            ps[:, q0 : q0 + qw],
                            lhsT,
                            rhs_ap(c0, c0 + qw),
                            start=True,
                            stop=True,
                        )
                        q0 += qw
                    nc.scalar.activation(
                        out=E[:, so : so + L],
                        in_=ps[:, 0:L],
                        func=AF.Exp,
                        scale=TEMP_SCALE,
                        **(
                            {"accum_out": acc[:, ROFF[m] + si : ROFF[m] + si + 1]}
                            if si in act_si
                            else {}
                        ),
                    )
                    if si not in act_si:
                        # row sums for the smaller slices ride on DVE instead
                        # of costing ACT an accumulator read
                        nc.vector.tensor_reduce(
                            out=acc[:, ROFF[m] + si : ROFF[m] + si + 1],
                            in_=E[:, so : so + L],
                            axis=AX.X,
                            op=OP.add,
                        )
                    if m == 7:
                        # final m-tile: drain everything per-slice to shrink
                        # the serial tail; m=6's colsums slot in right after
                        # the first fill so they stay off the tail
                        if si == 0 and pending is not None:
                            emit_colsums(*pending)
                            pending = None
                        for k in range(5):
                            cs0 = k * 1024 + m * 128 if k < 4 else 4096
                            if so <= cs0 and cs0 + 128 <= so + L:
                                emit_extract(m, E, k)
                        if so <= 4096 < so + L:
                            emit_corr(m, E)
                        emit_colsums(m, E, win=(so, so + L))
                    so += L

                if m < 7:
                    for k in range(5):
                        emit_extract(m, E, k)
                    emit_corr(m, E)
                    # previous m-tile's colsums go on the PE queue here, so
                    # they execute in the shadow of this m-tile's exps instead
                    # of delaying the next m-tile's PSUM fills
                    if pending is not None:
                        emit_colsums(*pending)
                    pending = (m, E)

            # drain CS -> SBUF -> DRAM; ship accumulators
            cs_sb = P.tile([8, 512], f32, tag="cs_sb")
            nc.scalar.activation(out=cs_sb[:], in_=CS[0:8, :], func=AF.Copy)
            nc.sync.dma_start(out=rows_out[:, :], in_=acc[:, 0:NSL])
            nc.sync.dma_start(out=g_out[:, :], in_=acc[:, NSL : NSL + 48])
            nc.sync.dma_start(out=cols_out[:, :], in_=cs_sb[:])

    nc.compile()
    return nc


def _get_nc():
    if "nc" not in _NC_CACHE:
        _NC_CACHE["nc"] = _build_nc()
    return _NC_CACHE["nc"]


def _host_prep(sim):
    import ml_dtypes

    x = np.asarray(sim, dtype=np.float64).reshape(K, D)
    z = (x / np.maximum(np.linalg.norm(x, axis=1, keepdims=True), 1e-8)).astype(
        np.float32
    )
    ident = np.eye(128, dtype=ml_dtypes.bfloat16)
    ind = np.zeros((128, 64), dtype=ml_dtypes.bfloat16)
    for k in range(8):
        ind[:, 8 * k + k] = 1
    in_maps = []
    for c in range(NCORES):
        ztc = np.ascontiguousarray(
            np.roll(z, -c * BS, axis=0)[:ZLOC].T.astype(ml_dtypes.bfloat16)
        )
        in_maps.append({"zt": ztc, "ident": ident, "ind": ind})
    return in_maps


def _assemble(results):
    """Gather/unshard: combine per-core partial sums into the scalar loss."""
    F = np.zeros(K, np.float64)
    G = np.zeros((K, 8), np.float64)
    li = np.arange(128)
    for c in range(NCORES):
        rows = np.asarray(results[c]["rows_out"], np.float64)
        g = np.asarray(results[c]["g_out"], np.float64)
        cols = np.asarray(results[c]["cols_out"], np.float64)
        for m in range(8):
            gr = c * BS + m * 128 + li  # global rows
            F[gr] += rows[:, ROFF[m] : ROFF[m] + len(SLICES[m])].sum(1)
            F[gr] -= g[:, 6 * m + 5]  # B4 diag-subtile double count
            for k in range(5):
                bc = (c + k) % 8
                G[gr, bc] = g[:, 6 * m + k]
                G[bc * BS + m * 128 + li, c] = g[:, 6 * m + k]
        for k in range(8):
            gcols = (c * BS + 1024 + 512 * k + np.arange(512)) % K
            F[gcols] += cols[k]
    P = G.sum(1)
    A = F - P
    idx = np.arange(K)
    mask = np.ones((K, 8), bool)
    mask[idx, idx // BS] = False
    Epos = G[mask].reshape(K, 7)
    L = np.log(Epos + A[:, None]) - np.log(Epos)
    return np.float32(L.sum() / (K * 7))


def kernel(sim: np.ndarray, _want_results: bool = False, _trace: bool = False):
    in_maps = _host_prep(sim)
    nc = _get_nc()
    from concourse.bass_utils import run_bass_kernel_spmd

    res = run_bass_kernel_spmd(nc, in_maps, list(range(NCORES)), trace=_trace)
    loss = _assemble(res.results)
    if _want_results:
        return loss, res
    return loss


if __name__ == "__main__":
    nc = _build_nc()
    print("build OK")
